# revision 19
# baseline (speedup 1.0000x reference)
"""GraphSAGE layer on 8 Trainium2 NeuronCores — v2 (narrow one-hot).

Strategy (1D graph partitioning, dst-major):
  - Nodes sharded across 8 cores (6250 dst rows each).  Edges bucketed to
    the core owning their destination and sorted by
    (tile, 32-dst window, src-half, src).  The full feature table is
    replicated in DRAM as bf16 rows padded to 128 cols (256 B stride).
  - Source rows fetched with SWDGE `dma_gather` (one descriptor per edge
    slot), round-robined over 4 queues; indices are int16 so the table is
    split at row 32768 (A/B halves).  Edge slots are padded per
    (tile, window, half) run to 128-multiples with idx 0 (row-0 reads are
    DRAM-row-buffer friendly); block structure is the max over the 8
    cores so one program serves all (SPMD).
  - One-hot S blocks are only [128 x 32] (the dst window), precomputed on
    host in bf16 — 4x less HBM traffic and 4x fewer matmul rhs columns
    than full-width [128 x 128] one-hots.
  - Per dst tile, PSUM nt[64, 128] accumulates
        nt[:, w*32:(w+1)*32] += G_blk.T @ S_blk
    per window accumulation group (start/stop per window).
  - Final linear: comb = [featT; nt] (bf16), one [128,128]x[128,64]
    matmul per tile.  L2 normalization is batched per 7 tiles:
    DVE tensor_tensor_reduce (square+sum fused) per tile, one ACT
    sqrt(+eps) + DVE reciprocal per group, ACT Copy per-partition scale.
"""

import sys

if "/opt/trn_rl_repo" not in sys.path:
    sys.path.insert(0, "/opt/trn_rl_repo")

import numpy as np
import ml_dtypes

import concourse.bacc as bacc
import concourse.tile as tile
from concourse import mybir
from concourse.bass_utils import run_bass_kernel_spmd

BF16 = ml_dtypes.bfloat16

N_NODES = 50000
N_EDGES = 800000
D = 64
DP = 128  # padded bf16 row length (256 B stride for the gather)
C = 8
NPC = N_NODES // C  # 6250
P = 128
T = (NPC + P - 1) // P  # 49 dst tiles/core
LAST_ROWS = NPC - (T - 1) * P  # 106
WIN = 32  # one-hot window width (dst cols per S block)
WPT = P // WIN  # 4 windows per tile
SPLIT = 32768  # int16 index limit for dma_gather
CB = 8  # max blocks per gather call (1024 idx = per-queue ring cap)
NQ = 4  # SWDGE queues
GRP = 7  # tiles per normalization batch (49 = 7*7)

_last_results = None


def _dma_gather_half(gp, out_ap, in_ap, idxs_ap, num_idxs, elem_size, queue_num):
    """dma_gather (non-transpose, DRAM source) with elem_size_bytes that is
    not a multiple of 256 B.  Mirrors BassGpSimd.dma_gather minus the
    256 B-elem assert — that alignment is only required by the transpose
    path; the non-transpose ucode emits one descriptor of elem_size_bytes
    per index with a row stride that must be 256 B aligned."""
    from concourse import ap_utils
    from concourse.bass import exact_div, round_up_to_multiple

    assert idxs_ap.dtype == mybir.dt.int16
    assert in_ap.dtype == out_ap.dtype
    elem_step = in_ap.ap[0][0]
    stride_bytes = elem_step * mybir.dt.size(in_ap.dtype)
    stride_bytes_256 = exact_div(stride_bytes, 256)
    assert ap_utils.ap_is_contiguous(out_ap.ap[1:])
    assert ap_utils.ap_is_contiguous(idxs_ap.ap[1:])
    assert out_ap.ap[0][1] * out_ap.ap[1][1] == round_up_to_multiple(num_idxs, 128)
    assert out_ap.ap[-1][1] == elem_size

    _in_ap = gp.lower_ap_dma(in_ap, for_custom_bir_dma=True)
    _idxs_ap = gp.lower_ap(idxs_ap)
    _out_ap = gp.lower_ap(out_ap)
    return gp.add_instruction(
        mybir.InstDMAGatherAnt(
            name=gp.bass.get_next_instruction_name(),
            ins=[
                *_in_ap,
                _idxs_ap,
                gp.lower_val_access(gp.to_reg(num_idxs)),
            ],
            outs=[_out_ap],
            transpose=False,
            num_idxs=num_idxs,
            elem_size=elem_size,
            stride_bytes_256=stride_bytes_256,
            gen_mode=0,
            single_packet=True,
            queue_num=queue_num,
            sbuf_tokens_per_rank=0,
            sbuf_free_dim_per_rank=0,
            sbuf_free_dim_pad_per_rank=0,
            sbuf_byte_offset=0,
        )
    )


def _prep(edge_src, edge_dst, edge_weight):
    """Per-core layouts: wrapped int16 gather indices and bf16 narrow
    one-hot S blocks, plus the (core-max, uniform) block structure."""
    src = edge_src.astype(np.int64)
    dst = edge_dst.astype(np.int64)
    w = edge_weight.astype(np.float32)

    cid = dst // NPC
    loc = dst - cid * NPC
    tid = loc // P
    win = (loc % P) // WIN
    col = loc % P - win * WIN
    half = (src >= SPLIT).astype(np.int64)

    seg = ((cid * T + tid) * WPT + win) * 2 + half
    order = np.argsort(seg * N_NODES + src, kind="stable")
    src, w, cid, tid, win, col, half, seg = (
        a[order] for a in (src, w, cid, tid, win, col, half, seg)
    )

    counts = np.bincount(seg, minlength=C * T * WPT * 2).reshape(C, T, WPT, 2)
    nb = (counts.max(axis=0) + P - 1) // P  # [T, WPT, 2] blocks per run
    aw = nb[:, :, 0]
    bw = nb[:, :, 1]
    nbA = aw.sum(axis=1)  # [T]
    nbB = bw.sum(axis=1)
    nbt = nbA + nbB
    tb0 = np.concatenate([[0], np.cumsum(nbt)])
    TOTB = int(tb0[-1])

    zcol = np.zeros((T, 1), np.int64)
    acum = np.concatenate([zcol, np.cumsum(aw, axis=1)], axis=1)
    bcum = np.concatenate([zcol, np.cumsum(bw, axis=1)], axis=1)
    seqoff = np.concatenate([zcol, np.cumsum(aw + bw, axis=1)], axis=1)
    st0 = tb0  # S stream shares the block count

    seg_counts = counts.reshape(-1)
    seg_starts = np.concatenate([[0], np.cumsum(seg_counts)])
    j = np.arange(len(src)) - np.repeat(seg_starts[:-1], seg_counts)
    jb = j // P
    part = j % P

    bufpos = np.where(half == 0, acum[tid, win] + jb, nbA[tid] + bcum[tid, win] + jb)
    gpos = tb0[tid] + bufpos

    # Pad slots reuse the last real src of their (c,t,win,half) run: the row
    # was just read (DRAM row-buffer hit) and pads spread across banks
    # instead of all hammering row 0.  Runs in buffer order per core:
    # per tile all A runs (win 0..3) then all B runs.
    idx16 = (src - half * SPLIT).astype(np.int16)
    seg_last = np.zeros(C * T * WPT * 2, np.int16)
    has = seg_counts > 0
    seg_last[has] = idx16[seg_starts[1:][has] - 1]
    seg_last4 = seg_last.reshape(C, T, WPT, 2)
    slot_val = np.empty((C, TOTB * P), np.int16)
    nbrun = nb.transpose(2, 0, 1)  # [2, T, WPT] -> index [half, t, win]
    for c in range(C):
        vals = []
        for t in range(T):
            for hf in (0, 1):
                for wn in range(WPT):
                    nbr = int(nb[t, wn, hf])
                    if nbr:
                        vals.append(
                            np.full(nbr * P, seg_last4[c, t, wn, hf], np.int16)
                        )
        slot_val[c] = np.concatenate(vals)
    cslot = gpos * P + part
    slot_val[cid, cslot] = idx16
    # wrap: slot s -> (col s//16, row s%16)
    idxw = np.ascontiguousarray(
        slot_val.reshape(C, TOTB * 8, 16).transpose(0, 2, 1)
    )

    # S stream in g-buffer order: S column block index == bufpos
    s_all = np.zeros((C, P, TOTB * WIN), BF16)
    s_all[cid, part, (tb0[tid] + bufpos) * WIN + col] = w.astype(BF16)

    return idxw, s_all, nb, tb0


def _build(nb, tb0, has_bias, debug=False):
    nc = bacc.Bacc(num_swdge_queues=NQ, dynamic_dma_scratch_size=32768)
    f32 = mybir.dt.float32
    bf16 = mybir.dt.bfloat16
    i16 = mybir.dt.int16

    aw = nb[:, :, 0]
    bw = nb[:, :, 1]
    nbA = aw.sum(axis=1)
    nbt = nbA + bw.sum(axis=1)
    zcol = np.zeros((T, 1), np.int64)
    acum = np.concatenate([zcol, np.cumsum(aw, axis=1)], axis=1)
    bcum = np.concatenate([zcol, np.cumsum(bw, axis=1)], axis=1)
    seqoff = np.concatenate([zcol, np.cumsum(aw + bw, axis=1)], axis=1)
    TOTB = int(tb0[-1])
    nbmax = int(nbt.max())

    featb = nc.declare_dram_parameter("featb", [N_NODES, DP], bf16, isOutput=False)
    idxw = nc.declare_dram_parameter("idxw", [P, TOTB * 8], i16, isOutput=False)
    s_all = nc.declare_dram_parameter("s_all", [P, TOTB * WIN], bf16, isOutput=False)
    featT = nc.declare_dram_parameter("featT", [D, T * P], bf16, isOutput=False)
    wt = nc.declare_dram_parameter("wt", [2 * D, D], bf16, isOutput=False)
    biasb = nc.declare_dram_parameter("biasb", [P, D], f32, isOutput=False)
    out = nc.declare_dram_parameter("out", [NPC, D], f32, isOutput=True)
    if debug:
        dbg_comb = nc.declare_dram_parameter(
            "dbg_comb", [P, T * P], mybir.dt.bfloat16, isOutput=True
        )
        dbg_po = nc.declare_dram_parameter("dbg_po", [P, T * D], f32, isOutput=True)
        dbg_g = nc.declare_dram_parameter("dbg_g", [P, nbmax * D], bf16, isOutput=True)
        dbg_s = nc.declare_dram_parameter("dbg_s", [P, nbmax * WIN], bf16, isOutput=True)

    qrr = [0]

    def next_q():
        q = qrr[0]
        qrr[0] = (q + 1) % NQ
        return q

    with tile.TileContext(nc) as tc:
        with (
            tc.tile_pool(name="singles", bufs=1) as singles,
            tc.tile_pool(name="gpool", bufs=12) as gpool,
            tc.tile_pool(name="spool", bufs=8) as spool,
            tc.tile_pool(name="cpool", bufs=4) as cpool,
            tc.tile_pool(name="opool", bufs=3) as opool,
            tc.tile_pool(name="onp", bufs=4) as onp,
            tc.tile_pool(name="stat", bufs=6) as stat,
            tc.tile_pool(name="pnT", bufs=3, space="PSUM") as pnT,
            tc.tile_pool(name="pout", bufs=3, space="PSUM") as pout,
        ):
            idx_sb = singles.tile([P, TOTB * 8], i16)
            wt_sb = singles.tile([2 * D, D], bf16)
            bias_sb = singles.tile([P, D], f32)
            eps_sb = singles.tile([P, 1], f32)
            # idx chunks: first tiles' indices land first
            cuts = [0, int(tb0[4]) * 8, int(tb0[16]) * 8, int(tb0[32]) * 8, TOTB * 8]
            for c0, c1 in zip(cuts[:-1], cuts[1:]):
                nc.sync.dma_start(out=idx_sb[:, c0:c1], in_=idxw[:, c0:c1])
            nc.sync.dma_start(out=wt_sb[:], in_=wt[:])
            if has_bias:
                nc.sync.dma_start(out=bias_sb[:], in_=biasb[:])
            nc.vector.memset(eps_sb[:], 1e-24)

            for t in range(T):
                nA, nT_ = int(nbA[t]), int(nbt[t])
                g = gpool.tile([P, nbmax * D], bf16, tag="g")
                for base_tbl, b0, b1 in ((0, 0, nA), (SPLIT, nA, nT_)):
                    nblk = b1 - b0
                    ncalls = (nblk + CB - 1) // CB
                    splits = [b0 + nblk * i // ncalls for i in range(ncalls + 1)]
                    for k0, k1 in zip(splits[:-1], splits[1:]):
                        gb = int(tb0[t]) + k0
                        _dma_gather_half(
                            nc.gpsimd,
                            out_ap=g[:, k0 * D : k1 * D].rearrange(
                                "p (n e) -> p n e", e=D
                            ),
                            in_ap=featb[base_tbl:, :],
                            idxs_ap=idx_sb[:, gb * 8 : (gb + (k1 - k0)) * 8],
                            num_idxs=(k1 - k0) * P,
                            elem_size=D,
                            queue_num=next_q(),
                        )
                s = spool.tile([P, nbmax * WIN], bf16, tag="s")
                nc.sync.dma_start(
                    out=s[:, : nT_ * WIN],
                    in_=s_all[:, int(tb0[t]) * WIN : (int(tb0[t]) + nT_) * WIN],
                )
                if debug and t == 0:
                    nc.sync.dma_start(out=dbg_g[:, : nT_ * D], in_=g[:, : nT_ * D])
                    nc.sync.dma_start(out=dbg_s[:, : nT_ * WIN], in_=s[:, : nT_ * WIN])
                # ONE accumulation group per tile (start=True marks the whole
                # 2 KB PSUM zero region pending-zero; each window slice is
                # lazily initialized by its first matmul).  Matmuls walk the
                # g buffer in order — all A blocks first — so the PE can chew
                # the A section while the B gather call still drains.
                nt = pnT.tile([D, P], f32)
                wins = [wn for wn in range(WPT) for _ in range(int(aw[t, wn]))] + [
                    wn for wn in range(WPT) for _ in range(int(bw[t, wn]))
                ]
                for bp, wn in enumerate(wins):
                    nc.tensor.matmul(
                        out=nt[:, wn * WIN : (wn + 1) * WIN],
                        lhsT=g[:, bp * D : (bp + 1) * D],
                        rhs=s[:, bp * WIN : (bp + 1) * WIN],
                        start=(bp == 0),
                        stop=(bp == nT_ - 1),
                    )
                comb = cpool.tile([P, P], bf16, tag="comb")
                nc.sync.dma_start(out=comb[:D, :], in_=featT[:, t * P : (t + 1) * P])
                nc.vector.tensor_copy(out=comb[D:, :], in_=nt[:])
                po = pout.tile([P, D], f32)
                nc.tensor.matmul(
                    out=po[:], lhsT=comb[:], rhs=wt_sb[:], start=True, stop=True
                )
                if debug:
                    nc.sync.dma_start(
                        out=dbg_comb[:, t * P : (t + 1) * P], in_=comb[:]
                    )
                    dpo = cpool.tile([P, D], f32, tag="dpo")
                    nc.vector.tensor_copy(out=dpo[:], in_=po[:])
                    nc.sync.dma_start(
                        out=dbg_po[:, t * D : (t + 1) * D], in_=dpo[:]
                    )
                if has_bias:
                    o = opool.tile([P, D], f32, tag="o")
                    nc.vector.tensor_add(out=o[:], in0=po[:], in1=bias_sb[:])
                    osrc = o
                else:
                    osrc = po
                sq = opool.tile([P, D], f32, tag="sq")
                ssum1 = stat.tile([P, 1], f32, tag="ssum")
                nc.scalar.activation(
                    out=sq[:],
                    in_=osrc[:],
                    func=mybir.ActivationFunctionType.Square,
                    accum_out=ssum1[:],
                )
                nrm = stat.tile([P, 1], f32, tag="nrm")
                nc.scalar.activation(
                    out=nrm[:],
                    in_=ssum1[:],
                    func=mybir.ActivationFunctionType.Sqrt,
                    bias=eps_sb[:],
                )
                rin = stat.tile([P, 1], f32, tag="rin")
                nc.vector.reciprocal(out=rin[:], in_=nrm[:])
                on = onp.tile([P, D], f32, tag="on")
                nc.scalar.activation(
                    out=on[:],
                    in_=osrc[:],
                    func=mybir.ActivationFunctionType.Copy,
                    scale=rin[:],
                )
                rows = LAST_ROWS if t == T - 1 else P
                # out-store on the ACT HWDGE queue: Sync's FIFO then only
                # carries loads, so tile t's store can't block tile t+1's
                # S/featT loads (HWDGE DMAs are FIFO per issuing engine).
                nc.scalar.dma_start(out=out[t * P : t * P + rows, :], in_=on[:rows, :])

    nc.compile()
    return nc


def kernel(features, edge_src, edge_dst, edge_weight, W=None, b=None, _cache={}, **kw):
    global _last_results
    features = np.ascontiguousarray(features, dtype=np.float32)
    edge_src = np.ascontiguousarray(edge_src, dtype=np.int32)
    edge_dst = np.ascontiguousarray(edge_dst, dtype=np.int32)
    edge_weight = np.ascontiguousarray(edge_weight, dtype=np.float32)
    Wm = np.ascontiguousarray(W, dtype=np.float32)
    b = np.ascontiguousarray(b, dtype=np.float32)

    idxw, s_all, nb, tb0 = _prep(edge_src, edge_dst, edge_weight)
    has_bias = bool(np.any(b != 0))

    featb = np.zeros((N_NODES, DP), BF16)
    featb[:, :D] = features.astype(BF16)
    featT = features.T.astype(BF16)
    featT_pad = np.zeros((C, D, T * P), BF16)
    for c in range(C):
        featT_pad[c, :, :NPC] = featT[:, c * NPC : (c + 1) * NPC]
    wt = np.ascontiguousarray(Wm.T).astype(BF16)
    biasb = np.ascontiguousarray(np.broadcast_to(b, (P, D))).astype(np.float32)

    key = ("v2", WIN, CB, has_bias, nb.tobytes())
    if key not in _cache:
        _cache.clear()
        _cache[key] = _build(nb, tb0, has_bias)
    nc = _cache[key]

    in_maps = [
        {
            "featb": featb,
            "idxw": np.ascontiguousarray(np.tile(idxw[c], (8, 1))),
            "s_all": np.ascontiguousarray(s_all[c]),
            "featT": featT_pad[c],
            "wt": wt,
            "biasb": biasb,
        }
        for c in range(C)
    ]
    import os

    trace = bool(os.environ.get("GS_TRACE"))
    if trace:
        try:
            import antenv.axon_hooks  # noqa: F401  (profiling-only dep)
        except ImportError:
            trace = False
    res = run_bass_kernel_spmd(nc, in_maps, core_ids=list(range(C)), trace=trace)
    _last_results = res
    out = np.concatenate([res.results[c]["out"] for c in range(C)], axis=0)
    return out.astype(np.float32)
